# revision 14
# baseline (speedup 1.0000x reference)
"""Bass/Trainium2 kernel for 4-layer LIF SNN (nn_Net_63531156242917).

Data-parallel over 8 NeuronCores: batch 256 -> 32 per core, weights replicated.
Per core: layer-major processing; matmuls batched over all 50 timesteps
(the time recurrence is elementwise-only), LIF dynamics on the vector engine.

Layouts (per core, Bl=32, BT=50*32=1600):
  activations (compute): [128 part = h_inner, kt = h_tile, bt = t*32+b]
  layer1 GEMM: native fp32 (lhsT = W1.T, rhs = x.T built via PE transposes)
  layers2-4 GEMM: lhsT = W.T as bf16 hi+lo (2-pass PSUM accum), rhs = spikes fp8
  outputs: PE-transposed back to [t, b, h] then DMA'd with 4KB-contiguous runs
"""
import sys
sys.path.insert(0, "/opt/trn_rl_repo")
import os
import numpy as np

import concourse.bass as bass
from concourse import bacc
import concourse.mybir as mybir
import concourse.tile as tile
from concourse.bass_utils import run_bass_kernel_spmd
from concourse.masks import make_identity

F32 = mybir.dt.float32
BF16 = mybir.dt.bfloat16
FP8 = mybir.dt.float8e4
ALU = mybir.AluOpType

P = 128
T, B, D, H, O = 50, 256, 768, 2048, 10
NCORES = 8
BL = B // NCORES          # 32
BT = T * BL               # 1600
KT_D = D // P             # 6
KT_H = H // P             # 16
MT_H = KT_H               # 16 output tiles for H layers
MH = MT_H // 2            # 8 mtiles per half
TC = 8                    # timesteps per chunk
CHUNKS = [(t0, min(TC, T - t0)) for t0 in range(0, T, TC)]  # [(0,8)...(48,2)]


def build(beta, th):
    """beta, th: lists of 4 python floats (clipped beta, thresholds)."""
    nc = bacc.Bacc(None, target_bir_lowering=False)

    # ---------------- I/O ----------------
    x_in = nc.dram_tensor("x", [BL, T, D], F32, kind="ExternalInput")
    Wd, bd = {}, {}
    for li, (od, idd) in enumerate([(H, D), (H, H), (H, H), (O, H)], start=1):
        Wd[li] = nc.dram_tensor(f"W{li}", [od, idd], F32, kind="ExternalInput")
        bd[li] = nc.dram_tensor(f"b{li}", [od], F32, kind="ExternalInput")
    outs = {}
    for li in (1, 2, 3):
        for pre in ("spk", "mem", "cur"):
            outs[(pre, li)] = nc.dram_tensor(f"{pre}{li}", [T, BL, H], F32, kind="ExternalOutput")
    for pre in ("spk", "mem", "cur"):
        outs[(pre, 4)] = nc.dram_tensor(f"{pre}4", [T, BL, O], F32, kind="ExternalOutput")

    with tile.TileContext(nc) as tc:
        with (
            tc.tile_pool(name="persist", bufs=1) as pp,
            tc.tile_pool(name="dramp", bufs=1, space="DRAM") as dp,
            tc.tile_pool(name="tp_psum", bufs=4, space="PSUM") as tpp,
        ):
            # internal HBM: pre-transposed weights and x.T (dram pool => dep tracking)
            w_hbm = {}
            for li in (2, 3):
                w_hbm[(li, "hi")] = dp.tile([P, KT_H, H], BF16, name=f"w{li}t_hi")
                w_hbm[(li, "lo")] = dp.tile([P, KT_H, H], BF16, name=f"w{li}t_lo")
            w1t_hbm = dp.tile([P, KT_D, H], F32, name="w1t_hbm")
            xt_hbm = dp.tile([P, KT_D, BT], F32, name="xt_hbm")
            ident = pp.tile([P, P], F32, name="ident")
            make_identity(nc, ident[:])

            # biases: [128, kt] for H layers; [10, 1] for L4
            bias_sb = {}
            for li in (1, 2, 3):
                t_ = pp.tile([P, KT_H], F32, name=f"bias{li}")
                nc.sync.dma_start(out=t_[:], in_=bd[li].rearrange("(kt p) -> p kt", p=P))
                bias_sb[li] = t_
            bias4 = pp.tile([O, 1], F32, name="bias4")
            nc.sync.dma_start(out=bias4[:, 0], in_=bd[4][:])

            # spike buffers (fp8, exact 0/1): s{l}T [128, kt, BT]
            sT = {1: pp.tile([P, KT_H, BT], FP8, name="s1T"),
                  2: pp.tile([P, KT_H, BT], FP8, name="s2T"),
                  3: pp.tile([P, KT_H, BT], FP8, name="s3T")}

            u_tmp = pp.tile([P, MH, BL], F32, name="u_tmp")
            u4_tmp = pp.tile([O, BL], F32, name="u4_tmp")

            # W4T: [128, 16, 10] bf16 hi/lo (prepped below)
            w4t_hi = pp.tile([P, KT_H, O], BF16, name="w4t_hi")
            w4t_lo = pp.tile([P, KT_H, O], BF16, name="w4t_lo")

            # ---------------- prep: W1T, W2T, W3T, W4T ----------------
            # (pool stays open through layer 1 so W2/W3 prep overlaps L1 compute)
            prep_ctx = tc.tile_pool(name="prep", bufs=1)
            prep = prep_ctx.__enter__()
            if True:
                # W1 -> W1T (fp32, PE transpose) -> HBM
                for pt in range(MT_H):
                    nat = prep.tile([P, D], F32, name="w1nat", tag="w1nat")
                    nc.sync.dma_start(out=nat[:], in_=Wd[1][pt * P:(pt + 1) * P, :])
                    asm = prep.tile([P, KT_D, P], F32, name="w1asm", tag="w1asm")
                    for j in range(KT_D):
                        pt_ps = tpp.tile([P, P], F32, name="w1tp", tag="tp")
                        nc.tensor.transpose(pt_ps[:], nat[:, j * P:(j + 1) * P], ident[:])
                        nc.vector.tensor_copy(asm[:, j, :], pt_ps[:])
                    nc.sync.dma_start(out=w1t_hbm[:, :, pt * P:(pt + 1) * P], in_=asm[:])

                # W2/W3 -> decompose bf16 hi/lo -> DMA-transpose -> HBM
                HH = H // 2
                for li in (2, 3):
                    for pt in range(KT_H):
                        for ch in range(2):
                            nat = prep.tile([P, HH], F32, name=f"wnat", tag="wnat")
                            nc.sync.dma_start(
                                out=nat[:],
                                in_=Wd[li][pt * P:(pt + 1) * P, ch * HH:(ch + 1) * HH])
                            hi = prep.tile([P, HH], BF16, name="whi", tag="whi")
                            lo = prep.tile([P, HH], BF16, name="wlo", tag="wlo")
                            nc.vector.tensor_copy(hi[:], nat[:])
                            nc.vector.scalar_tensor_tensor(
                                out=lo[:], in0=hi[:], scalar=-1.0, in1=nat[:],
                                op0=ALU.mult, op1=ALU.add)
                            for part, t_ in (("hi", hi), ("lo", lo)):
                                asm = prep.tile([P, KT_H // 2, P], BF16, name="wasm", tag="wasm")
                                nc.sync.dma_start_transpose(out=asm[:], in_=t_[:])
                                nc.sync.dma_start(
                                    out=w_hbm[(li, part)][:, ch * 8:(ch + 1) * 8,
                                                          pt * P:(pt + 1) * P],
                                    in_=asm[:])

                # W4 [10, 2048] -> W4T [128, 16, 10] via PE transpose
                w4f = prep.tile([P, KT_H, O], F32, name="w4f")
                for ch in range(2):
                    w4nat = prep.tile([O, HH], F32, name="w4nat", tag="w4nat")
                    nc.sync.dma_start(out=w4nat[:], in_=Wd[4][:, ch * HH:(ch + 1) * HH])
                    for j in range(KT_H // 2):
                        pt_ps = tpp.tile([P, P], F32, name="w4tp", tag="tp")
                        nc.tensor.transpose(pt_ps[:, :O], w4nat[:, j * P:(j + 1) * P],
                                            ident[:O, :O])
                        nc.vector.tensor_copy(w4f[:, ch * 8 + j, :], pt_ps[:, :O])
                nc.vector.tensor_copy(w4t_hi[:], w4f[:])
                nc.vector.scalar_tensor_tensor(
                    out=w4t_lo[:], in0=w4t_hi[:], scalar=-1.0, in1=w4f[:],
                    op0=ALU.mult, op1=ALU.add)

            # ---------------- layers 1..3 ----------------
            for li in (1, 2, 3):
                kt_in = KT_D if li == 1 else KT_H
                fp32_layer = (li == 1)
                with (
                    tc.tile_pool(name=f"L{li}", bufs=1) as lp,
                    tc.tile_pool(name=f"L{li}psum", bufs=1, space="PSUM") as gp,
                ):
                    if fp32_layer:
                        w_sb = {"f32": lp.tile([P, KT_D, MH * P], F32, name="w1half", tag="whalf")}
                        xt_ring = [lp.tile([P, KT_D, TC * BL], F32, name=f"xtr{i}", tag=f"xtr{i}")
                                   for i in range(2)]
                        xnat = [lp.tile([P, 2, D], F32, name=f"xn{i}", tag=f"xn{i}")
                                for i in range(2)]
                    else:
                        w_sb = {"hi": lp.tile([P, KT_H, MH * P], BF16, name="whi_s", tag="whalf_hi"),
                                "lo": lp.tile([P, KT_H, MH * P], BF16, name="wlo_s", tag="whalf_lo")}
                    cur_sb = [lp.tile([P, MH, TC, BL], F32, name=f"cur{i}", tag=f"cur{i}")
                              for i in range(2)]
                    mem_hist = [lp.tile([P, MH, TC, BL], F32, name=f"memh{i}", tag=f"memh{i}")
                                for i in range(2)]
                    curT = lp.tile([P, 2, MH, P], F32, name="curT")
                    memT = lp.tile([P, 2, MH, P], F32, name="memT")
                    spkN = lp.tile([P, 2, MH, P], F32, name="spkN")

                    for half in range(2):
                        mt0 = half * MH
                        # stream in this half's weights
                        if fp32_layer:
                            nc.sync.dma_start(out=w_sb["f32"][:],
                                              in_=w1t_hbm[:, :, mt0 * P:(mt0 + MH) * P])
                        else:
                            for part in ("hi", "lo"):
                                nc.sync.dma_start(out=w_sb[part][:],
                                                  in_=w_hbm[(li, part)][:, :, mt0 * P:(mt0 + MH) * P])

                        for ci, (t0, tcn) in enumerate(CHUNKS):
                            ncols = tcn * BL
                            t4 = min(4, tcn)
                            nbt = (tcn * BL + P - 1) // P  # bt tiles (2 or 1)
                            # ---- rhs for layer1: x.T chunk (transpose on the fly / reload)
                            if fp32_layer:
                                xr = xt_ring[ci % 2]
                                if half == 0:
                                    xn = xnat[ci % 2]
                                    for ti in range(tcn):
                                        bt, t4i = divmod(ti, 4)
                                        nc.sync.dma_start(
                                            out=xn[t4i * BL:(t4i + 1) * BL, bt, :],
                                            in_=x_in[:, t0 + ti, :])
                                    for j in range(KT_D):
                                        for bt in range(nbt):
                                            pt_ps = tpp.tile([P, P], F32, name="xtp", tag="tp")
                                            nc.tensor.transpose(
                                                pt_ps[:, :t4 * BL],
                                                xn[:t4 * BL, bt, j * P:(j + 1) * P],
                                                ident[:t4 * BL, :t4 * BL])
                                            nc.vector.tensor_copy(
                                                xr[:, j, bt * P:bt * P + t4 * BL],
                                                pt_ps[:, :t4 * BL])
                                    nc.sync.dma_start(
                                        out=xt_hbm[:, :, t0 * BL:t0 * BL + ncols],
                                        in_=xr[:, :, :ncols])
                                else:
                                    nc.sync.dma_start(
                                        out=xr[:, :, :ncols],
                                        in_=xt_hbm[:, :, t0 * BL:t0 * BL + ncols])
                                rhs_all = xr
                                rhs_off = 0
                            else:
                                rhs_all = sT[li - 1]
                                rhs_off = t0 * BL

                            # ---- GEMM chunk -> psum
                            psum = gp.tile([P, MH, TC * BL], F32, name="gpsum", tag="gpsum")
                            for mt in range(MH):
                                out_ap = psum[:, mt, :ncols]
                                if fp32_layer:
                                    for kt in range(kt_in):
                                        nc.tensor.matmul(
                                            out_ap,
                                            w_sb["f32"][:, kt, mt * P:(mt + 1) * P],
                                            rhs_all[:, kt, rhs_off:rhs_off + ncols],
                                            start=(kt == 0), stop=(kt == kt_in - 1))
                                else:
                                    for pi, part in enumerate(("hi", "lo")):
                                        for kt in range(kt_in):
                                            nc.tensor.matmul(
                                                out_ap,
                                                w_sb[part][:, kt, mt * P:(mt + 1) * P],
                                                rhs_all[:, kt, rhs_off:rhs_off + ncols],
                                                start=(pi == 0 and kt == 0),
                                                stop=(pi == 1 and kt == kt_in - 1))

                            # ---- drain + bias -> cur_sb
                            cs = cur_sb[ci % 2]
                            nc.vector.scalar_tensor_tensor(
                                out=cs[:, :, :tcn, :].rearrange("p m t b -> p m (t b)"),
                                in0=psum[:, :, :ncols],
                                scalar=1.0,
                                in1=bias_sb[li][:, mt0:mt0 + MH, None].broadcast_to(
                                    [P, MH, ncols]),
                                op0=ALU.mult, op1=ALU.add)

                            # ---- LIF over timesteps in chunk
                            mh = mem_hist[ci % 2]
                            for ti in range(tcn):
                                tg = t0 + ti
                                c_ap = cs[:, :, ti, :]
                                m_ap = mh[:, :, ti, :]
                                if tg == 0:
                                    nc.vector.tensor_copy(m_ap, c_ap)
                                else:
                                    if ti == 0:
                                        prev_tcn = CHUNKS[ci - 1][1]
                                        mprev = mem_hist[(ci - 1) % 2][:, :, prev_tcn - 1, :]
                                    else:
                                        mprev = mh[:, :, ti - 1, :]
                                    sprev = sT[li][:, mt0:mt0 + MH,
                                                   (tg - 1) * BL:tg * BL]
                                    nc.vector.scalar_tensor_tensor(
                                        out=u_tmp[:], in0=sprev, scalar=-th[li - 1],
                                        in1=c_ap, op0=ALU.mult, op1=ALU.add)
                                    nc.vector.scalar_tensor_tensor(
                                        out=m_ap, in0=mprev, scalar=beta[li - 1],
                                        in1=u_tmp[:], op0=ALU.mult, op1=ALU.add)
                                nc.vector.tensor_scalar(
                                    out=sT[li][:, mt0:mt0 + MH, tg * BL:(tg + 1) * BL],
                                    in0=m_ap, scalar1=th[li - 1], scalar2=None,
                                    op0=ALU.is_gt)

                            # ---- transpose outputs back to [t, b, h] and DMA
                            for (src, dstT) in ((cs, curT), (mh, memT)):
                                for bt in range(nbt):
                                    tt = min(4, tcn - bt * 4)
                                    for mt in range(MH):
                                        pt_ps = tpp.tile([P, P], F32, name="otp", tag="tp")
                                        nc.tensor.transpose(
                                            pt_ps[:tt * BL, :],
                                            src[:, mt, bt * 4:bt * 4 + tt, :].rearrange(
                                                "p t b -> p (t b)"),
                                            ident[:])
                                        nc.vector.tensor_copy(
                                            dstT[:tt * BL, bt, mt, :], pt_ps[:tt * BL, :])
                            nc.vector.tensor_scalar(
                                out=spkN[:t4 * BL, :nbt, :, :],
                                in0=memT[:t4 * BL, :nbt, :, :],
                                scalar1=th[li - 1], scalar2=None, op0=ALU.is_gt)
                            for (pre, srcT) in (("cur", curT), ("mem", memT), ("spk", spkN)):
                                nc.sync.dma_start(
                                    out=outs[(pre, li)][t0:t0 + tcn, :, mt0 * P:(mt0 + MH) * P]
                                    .rearrange("(bt t4) b (mt hin) -> (t4 b) bt mt hin",
                                               t4=t4, hin=P),
                                    in_=srcT[:t4 * BL, :nbt, :, :])
                if li == 1:
                    prep_ctx.__exit__(None, None, None)

            # ---------------- layer 4 ----------------
            with (
                tc.tile_pool(name="L4", bufs=1) as lp,
                tc.tile_pool(name="L4psum", bufs=2, space="PSUM") as gp4,
            ):
                cur4 = [lp.tile([O, TC, BL], F32, name=f"cur4_{i}", tag=f"cur4_{i}")
                        for i in range(2)]
                mem4 = [lp.tile([O, TC, BL], F32, name=f"mem4_{i}", tag=f"mem4_{i}")
                        for i in range(2)]
                c4T = lp.tile([P, 2, O], F32, name="c4T")
                m4T = lp.tile([P, 2, O], F32, name="m4T")
                s4N = lp.tile([P, 2, O], F32, name="s4N")
                s4_sb = lp.tile([O, T * BL], F32, name="s4_sb")  # spikes fp32 (no next layer)

                for ci, (t0, tcn) in enumerate(CHUNKS):
                    ncols = tcn * BL
                    t4 = min(4, tcn)
                    nbt = (ncols + P - 1) // P
                    psum = gp4.tile([O, TC * BL], F32, name="g4psum", tag="g4psum")
                    for pi, wt_ in enumerate((w4t_hi, w4t_lo)):
                        for kt in range(KT_H):
                            nc.tensor.matmul(
                                psum[:, :ncols], wt_[:, kt, :],
                                sT[3][:, kt, t0 * BL:t0 * BL + ncols],
                                start=(pi == 0 and kt == 0),
                                stop=(pi == 1 and kt == KT_H - 1))
                    cs = cur4[ci % 2]
                    nc.vector.scalar_tensor_tensor(
                        out=cs[:, :tcn, :].rearrange("o t b -> o (t b)"),
                        in0=psum[:, :ncols],
                        scalar=1.0,
                        in1=bias4[:, 0:1].broadcast_to([O, ncols]),
                        op0=ALU.mult, op1=ALU.add)
                    mh = mem4[ci % 2]
                    for ti in range(tcn):
                        tg = t0 + ti
                        c_ap = cs[:, ti, :]
                        m_ap = mh[:, ti, :]
                        if tg == 0:
                            nc.vector.tensor_copy(m_ap, c_ap)
                        else:
                            if ti == 0:
                                prev_tcn = CHUNKS[ci - 1][1]
                                mprev = mem4[(ci - 1) % 2][:, prev_tcn - 1, :]
                            else:
                                mprev = mh[:, ti - 1, :]
                            sprev = s4_sb[:, (tg - 1) * BL:tg * BL]
                            nc.vector.scalar_tensor_tensor(
                                out=u4_tmp[:], in0=sprev, scalar=-th[3],
                                in1=c_ap, op0=ALU.mult, op1=ALU.add)
                            nc.vector.scalar_tensor_tensor(
                                out=m_ap, in0=mprev, scalar=beta[3],
                                in1=u4_tmp[:], op0=ALU.mult, op1=ALU.add)
                        nc.vector.tensor_scalar(
                            out=s4_sb[:, tg * BL:(tg + 1) * BL],
                            in0=m_ap, scalar1=th[3], scalar2=None, op0=ALU.is_gt)

                    for (src, dstT) in ((cs[:].rearrange("o t b -> o (t b)"), c4T),
                                        (mh[:].rearrange("o t b -> o (t b)"), m4T),
                                        (s4_sb[:, t0 * BL:t0 * BL + ncols], s4N)):
                        for bt in range(nbt):
                            tt = min(4, tcn - bt * 4)
                            pt_ps = tpp.tile([P, P], F32, name="o4tp", tag="tp")
                            nc.tensor.transpose(
                                pt_ps[:tt * BL, :O],
                                src[:, bt * P:bt * P + tt * BL],
                                ident[:O, :O])
                            nc.vector.tensor_copy(dstT[:tt * BL, bt, :], pt_ps[:tt * BL, :O])
                    for (pre, srcT) in (("cur", c4T), ("mem", m4T), ("spk", s4N)):
                        nc.sync.dma_start(
                            out=outs[(pre, 4)][t0:t0 + tcn, :, :]
                            .rearrange("(bt t4) b o -> (t4 b) bt o", t4=t4),
                            in_=srcT[:t4 * BL, :nbt, :])

    nc.finalize()
    return nc


_CACHE = {}


def _get_nc(beta, th):
    key = (tuple(beta), tuple(th))
    if key not in _CACHE:
        _CACHE[key] = build(list(beta), list(th))
    return _CACHE[key]


def kernel(**inputs):
    x = np.ascontiguousarray(np.asarray(inputs["x"], dtype=np.float32))
    beta_in = np.asarray(inputs["beta"], dtype=np.float32)
    th_in = np.asarray(inputs["threshold"], dtype=np.float32)
    beta = [float(min(max(b, 0.0), 1.0)) for b in beta_in]
    th = [float(v) for v in th_in]

    nc = _get_nc(beta, th)

    common = {}
    for li in (1, 2, 3, 4):
        common[f"W{li}"] = np.ascontiguousarray(np.asarray(inputs[f"W{li}"], np.float32))
        common[f"b{li}"] = np.ascontiguousarray(np.asarray(inputs[f"b{li}"], np.float32))
    in_maps = []
    for i in range(NCORES):
        m = dict(common)
        m["x"] = np.ascontiguousarray(x[i * BL:(i + 1) * BL])
        in_maps.append(m)

    trace = bool(int(os.environ.get("KERNEL_TRACE", "0")))
    res = run_bass_kernel_spmd(nc, in_maps, core_ids=list(range(NCORES)), trace=trace)
    if trace:
        kernel.last_exec_time_ns = res.exec_time_ns
        kernel.last_result = res

    def gather(name):
        return np.concatenate([res.results[i][name] for i in range(NCORES)], axis=1)

    return (gather("spk1"), gather("spk2"), gather("spk3"), gather("spk4"),
            gather("mem1"), gather("mem2"), gather("mem3"), gather("mem4"),
            gather("cur1"), gather("cur2"), gather("cur3"), gather("cur4"))
